# revision 37
# baseline (speedup 1.0000x reference)
"""Multi-head attention (B=8, S=1024, H=1024, NH=16) on 8 trn2 NeuronCores.

Data-parallel over batch: one batch element per core. v5 design (v4 +):

  - Q/K projections run as fp8e4 DoubleRow matmuls (two 128-deep k-tiles
    per instruction at 0.5 cycles/row). Wq/Wk are scaled by 16 host-side
    so their values clear the fp8 subnormal floor; the extra 256x on the
    scores is folded into the exp scale. query/key inputs are quantized
    to fp8 host-side. V and O projections stay bf16 (their quantization
    error would hit the output directly; Q/K noise is damped by softmax).
  - Inputs arrive host-pre-tiled so each tensor is 1-2 large DMAs
    ([P, HT*cols] layout), issued wq0-first so the first projection
    starts as soon as ~640KB has landed instead of after the whole
    input set.
  - Per head pair the AV matmuls write head A to PSUM partitions 0:64
    (lhsT [V_A | ones]) and head B to partitions 64:128 directly
    (lhsT [ones | zeros*63 | V_B]), so evacuation is two plain vector
    copies - no partition-shift DMA. Denominators: A's rides row 64
    (staged via ScalarE copy + 1-row DMA to partition 1), B's lands on
    partition 0 where the custom-DVE reciprocal can read it.
  - exp() split across engines: per head (1..7) jt0/jt1 go through a
    Schraudolph bit-trick exp (VectorE int32 affine, then the bitcast
    copy split half GpSimd / half VectorE), jt2..7 through ScalarE.
    Their AV matmuls are emitted last in the head (PSUM accumulation
    order is free between start/stop).
  - Output stored bf16 (cast to f32 host-side), final tile split in two
    so the last DMA hides behind the last matmuls.
"""

import math
from contextlib import ExitStack

import ml_dtypes
import numpy as np

import concourse.bass as bass  # noqa: F401
import concourse.mybir as mybir
import concourse.tile as tile
from concourse import bacc
from concourse.bass_utils import run_bass_kernel_spmd

B, S, H, NH = 8, 1024, 1024, 16
HD = H // NH  # 64
P = 128
HT = H // P  # 8
ST = S // P  # 8
NI = 512
IC = S // NI  # 2
VA = HD + 1  # 65
PW = VA + P  # 193: per-pair Vaug block [V_A|1_A | 1_B|0*63|V_B]
NEG = np.float32(-1e32)
SCALE = 1.0 / math.sqrt(H)
WS = 16.0  # host-side Wq/Wk scale for fp8
SCALE_EFF = SCALE / (WS * WS)

# 16-bit Schraudolph exp: round(s*128/ln2 + (127*128 - C16)) as int16 IS the
# bf16 bit pattern of ~exp(s) (rms err ~1.6%). One DVE op, no casts. Masked
# scores get bias -26000: the affine stays in [-31500, -20500], whose int16
# patterns bitcast to ~-1e-10..-1e-36 - effectively zero attention weight.
A16 = 128.0 / math.log(2)
C16 = 5.0
B16 = 127.0 * 128 - C16
SCH16_MASKED = -26000.0

BF = mybir.dt.bfloat16
F32 = mybir.dt.float32
I16 = mybir.dt.int16
FP8 = mybir.dt.float8e4
EXP = mybir.ActivationFunctionType.Exp
MUL = mybir.AluOpType.mult
ADD = mybir.AluOpType.add
DR = mybir.MatmulPerfMode.DoubleRow

_CACHE: dict = {}


def build_program(has_bias: bool):
    nc = bacc.Bacc(None, target_bir_lowering=False)

    xq_d = nc.declare_dram_parameter("xq", [P, HT * S], FP8, isOutput=False)
    xk_d = nc.declare_dram_parameter("xk", [P, HT * S], FP8, isOutput=False)
    xv_d = nc.declare_dram_parameter("xv", [P, HT * S], BF, isOutput=False)
    # wq/wk: [p, ot*H + kt*128 + c] = 16*W.T[kt*128+p, ot*128+c], fp8
    wq_d = nc.declare_dram_parameter("wq", [P, HT * H], FP8, isOutput=False)
    wk_d = nc.declare_dram_parameter("wk", [P, HT * H], FP8, isOutput=False)
    # wv/wo: [p, kt*H + c] = W.T[kt*128+p, c], bf16
    wv_d = nc.declare_dram_parameter("wv", [P, HT * H], BF, isOutput=False)
    wo_d = nc.declare_dram_parameter("wo", [P, HT * H], BF, isOutput=False)
    maskb_d = nc.declare_dram_parameter("maskb", [P, ST], F32, isOutput=False)
    sbias_d = nc.declare_dram_parameter("sbias", [P, ST], F32, isOutput=False)
    sel2_d = nc.declare_dram_parameter("sel2", [2, P], BF, isOutput=False)
    if has_bias:
        bqT_d = nc.declare_dram_parameter("bqT", [P, HT], F32, isOutput=False)
        bkT_d = nc.declare_dram_parameter("bkT", [P, HT], F32, isOutput=False)
        bvb_d = nc.declare_dram_parameter("bvb", [P, H], BF, isOutput=False)
        bob_d = nc.declare_dram_parameter("bob", [P, H], F32, isOutput=False)
    y_d = nc.declare_dram_parameter("y", [S, H], BF, isOutput=True)

    with tile.TileContext(nc) as tc, ExitStack() as ctx:
        sb = ctx.enter_context(tc.tile_pool(name="sb", bufs=1))
        ps = ctx.enter_context(tc.tile_pool(name="ps", bufs=1, space="PSUM"))

        # ---------- input DMAs, first-needed first ----------
        wq = sb.tile([P, HT * H], FP8, tag="wq", name="wq")
        nc.sync.dma_start(out=wq[:, 0:H], in_=wq_d[:, 0:H])
        xq = sb.tile([P, HT * S], FP8, tag="xq", name="xq")
        for qt in range(4):
            nc.sync.dma_start(
                out=xq[:, qt * 2 * S : (qt + 1) * 2 * S],
                in_=xq_d[:, qt * 2 * S : (qt + 1) * 2 * S],
            )
        wk = sb.tile([P, HT * H], FP8, tag="wk", name="wk")
        nc.sync.dma_start(out=wk[:, 0:H], in_=wk_d[:, 0:H])
        xk = sb.tile([P, HT * S], FP8, tag="xk", name="xk")
        nc.sync.dma_start(out=xk[:, 0 : 4 * S], in_=xk_d[:, 0 : 4 * S])
        nc.sync.dma_start(out=xk[:, 4 * S :], in_=xk_d[:, 4 * S :])

        maskb = sb.tile([P, ST], F32, tag="maskb")
        nc.sync.dma_start(out=maskb[:], in_=maskb_d[:])
        sbias = sb.tile([P, ST], F32, tag="sbias")
        nc.sync.dma_start(out=sbias[:], in_=sbias_d[:])
        sel2 = sb.tile([2, P], BF, tag="sel2")
        nc.sync.dma_start(out=sel2[:], in_=sel2_d[:])

        # V inputs stream before the remaining Q/K weights: the head-0
        # V projection is needed well before projections Q1../K1.. are.
        wvp = tc.alloc_tile_pool(name="wvp", bufs=1)
        xv = wvp.tile([P, HT * S], BF, tag="xv", name="xv")
        nc.sync.dma_start(out=xv[:, 0 : 4 * S], in_=xv_d[:, 0 : 4 * S])
        nc.sync.dma_start(out=xv[:, 4 * S :], in_=xv_d[:, 4 * S :])
        wv = wvp.tile([P, HT * H], BF, tag="wv", name="wv")
        nc.sync.dma_start(out=wv[:, 0 : 4 * H], in_=wv_d[:, 0 : 4 * H])
        nc.sync.dma_start(out=wv[:, 4 * H :], in_=wv_d[:, 4 * H :])
        xv3 = xv.rearrange("p (kt c) -> p kt c", c=S)
        wv3 = wv.rearrange("p (kt c) -> p kt c", c=H)

        nc.sync.dma_start(out=wq[:, H:], in_=wq_d[:, H:])
        nc.sync.dma_start(out=wk[:, H:], in_=wk_d[:, H:])
        if has_bias:
            bqT = sb.tile([P, HT], F32, tag="bqT")
            nc.sync.dma_start(out=bqT[:], in_=bqT_d[:])
            bkT = sb.tile([P, HT], F32, tag="bkT")
            nc.sync.dma_start(out=bkT[:], in_=bkT_d[:])
            bvb = sb.tile([P, H], BF, tag="bvb")
            nc.sync.dma_start(out=bvb[:], in_=bvb_d[:])
            bob = sb.tile([P, H], F32, tag="bob")
            nc.sync.dma_start(out=bob[:], in_=bob_d[:])

        xq3 = xq.rearrange("p (kt c) -> p kt c", c=S)
        xk3 = xk.rearrange("p (kt c) -> p kt c", c=S)
        wq4 = wq.rearrange("p (ot kt c) -> p ot kt c", ot=HT, c=P)
        wk4 = wk.rearrange("p (ot kt c) -> p ot kt c", ot=HT, c=P)

        QT = [sb.tile([P, S], BF, tag=f"QT{i}", name=f"QT{i}") for i in range(HT)]
        KT = [sb.tile([P, S], BF, tag=f"KT{i}", name=f"KT{i}") for i in range(HT)]
        Vaug = [
            sb.tile([P, ST * PW], BF, tag=f"Va{i}", name=f"Va{i}") for i in range(ST)
        ]
        OT = [sb.tile([P, S], BF, tag=f"OT{i}", name=f"OT{i}") for i in range(HT)]

        def proj_qk(x3, w4, out_tile, ot, bias_tile):
            pj = ps.tile([P, S], F32, tag="big", bufs=2, name="pj")
            for j in range(HT // 2):
                ks = slice(2 * j, 2 * j + 2)
                nc.tensor.matmul(
                    pj[:, 0:NI], w4[:, ot, ks, :], x3[:, ks, 0:NI],
                    start=(j == 0), stop=(j == HT // 2 - 1), perf_mode=DR,
                )
                nc.tensor.matmul(
                    pj[:, NI:S], w4[:, ot, ks, :], x3[:, ks, NI:S],
                    start=(j == 0), stop=(j == HT // 2 - 1), perf_mode=DR,
                )
            if has_bias:
                nc.vector.tensor_scalar_add(
                    out_tile[:], pj[:], bias_tile[:, ot : ot + 1]
                )
            else:
                nc.vector.tensor_copy(out_tile[:], pj[:])

        proj_qk(xq3, wq4, QT[0], 0, None if not has_bias else bqT)
        proj_qk(xk3, wk4, KT[0], 0, None if not has_bias else bkT)

        wo = None

        def v_proj_tile(st):
            va3 = Vaug[st].rearrange("p (pr w) -> p pr w", w=PW)
            nc.gpsimd.memset(va3[:, :, HD : HD + 2], 1.0)
            nc.gpsimd.memset(va3[:, :, HD + 2 : PW - HD], 0.0)
            pv = ps.tile([P, S], F32, tag="big", bufs=2, name="pv")
            for kt in range(HT):
                xs = xv3[:, kt, st * P : (st + 1) * P]
                nc.tensor.matmul(
                    pv[:, 0:NI], xs, wv3[:, kt, 0:NI],
                    start=(kt == 0), stop=(kt == HT - 1),
                )
                nc.tensor.matmul(
                    pv[:, NI:S], xs, wv3[:, kt, NI:S],
                    start=(kt == 0), stop=(kt == HT - 1),
                )
            pv4 = pv.rearrange("p (pr two c) -> p pr two c", two=2, c=HD)
            if has_bias:
                bv4 = bvb.rearrange("p (pr two c) -> p pr two c", two=2, c=HD)
                nc.vector.tensor_add(va3[:, :, 0:HD], pv4[:, :, 0, :], bv4[:, :, 0, :])
                nc.vector.tensor_add(
                    va3[:, :, PW - HD : PW], pv4[:, :, 1, :], bv4[:, :, 1, :]
                )
            else:
                # Head 0's phase 2: ScalarE takes the A halves (its exps ran
                # in phase 1), VectorE the B halves. GpSimd can't read PSUM.
                nc.scalar.copy(va3[:, :, 0:HD], pv4[:, :, 0, :])
                nc.vector.tensor_copy(va3[:, :, PW - HD : PW], pv4[:, :, 1, :])

        def scores_tile(ht, jt, ic):
            jc = slice(jt * P, (jt + 1) * P)
            cc = slice(ic * NI, (ic + 1) * NI)
            sc = ps.tile([P, S], F32, tag="big", bufs=2, name="sc")
            nc.tensor.matmul(
                sc[:, 0:NI], KT[ht][0:HD, jc], QT[ht][0:HD, cc],
                start=True, stop=True,
            )
            nc.tensor.matmul(
                sc[:, NI:S], KT[ht][HD:P, jc], QT[ht][HD:P, cc],
                start=True, stop=True,
            )
            return sc

        def exp_tile(sc, jt, eng, long_lived):
            tag, bufs = ("attL", 8) if long_lived else ("attn", 8)
            at = sb.tile([P, S], BF, tag=tag, bufs=bufs, name=tag)
            if eng == "S":
                nc.scalar.activation(
                    at[:], sc[:], EXP, bias=maskb[:, jt : jt + 1], scale=SCALE_EFF
                )
            else:  # H: one-op 16-bit Schraudolph on VectorE
                nc.vector.tensor_scalar(
                    out=at[:].bitcast(I16), in0=sc[:],
                    scalar1=A16 * SCALE_EFF,
                    scalar2=sbias[:, jt : jt + 1], op0=MUL, op1=ADD,
                )
            return at

        # ---------- attention ----------
        def emit_evac(ht, avA, avB, dsg_on_vector=True):
            # Three of the four PSUM evacuation reads ride ScalarE (whose
            # first exp of the following head isn't needed for ~5us); the
            # dsg_A staging row is deferred to VectorE behind the Schraudolph
            # TS ops (its consumer - the reciprocal - runs at jt5).
            nc.scalar.copy(OT[ht][0:HD, :], avA[0:HD, :])
            nc.scalar.copy(OT[ht][HD:P, :], avB[HD:P, :])
            dcf = sb.tile([2, S], F32, tag="dcf", bufs=2, name="dcf")
            nc.scalar.copy(dcf[0:1, :], avB[0:1, :])
            dsg = sb.tile([VA, S], F32, tag="dsg", bufs=2, name="dsg")
            if dsg_on_vector:
                return (ht, dcf, dsg, avA)
            nc.scalar.copy(dsg[HD:VA, :], avA[HD:VA, :])
            nc.sync.dma_start(out=dcf[1:2, :], in_=dsg[HD:VA, :])
            return (ht, dcf, None, None)

        def emit_dsg(pending):
            ht, dcf, dsg, avA = pending
            nc.vector.tensor_copy(dsg[HD:VA, :], avA[HD:VA, :])
            nc.sync.dma_start(out=dcf[1:2, :], in_=dsg[HD:VA, :])
            return (ht, dcf, None, None)

        def emit_normalize(pending, rt_tag="big", rt_bufs=2):
            ht, dcf = pending[0], pending[1]
            rcf = sb.tile([2, S], F32, tag="rcf", bufs=2, name="rcf")
            nc.vector.reciprocal_approx_fast(out=rcf[:], in_=dcf[:])
            rcb = sb.tile([2, S], BF, tag="rcb", bufs=2, name="rcb")
            nc.vector.tensor_copy(rcb[:], rcf[:])
            rt = ps.tile([P, S], F32, tag=rt_tag, bufs=rt_bufs, name="rt")
            for ic in range(IC):
                cc = slice(ic * NI, (ic + 1) * NI)
                nc.tensor.matmul(
                    rt[:, cc], sel2[:], rcb[:, cc], start=True, stop=True
                )
            nc.vector.tensor_mul(OT[ht][:], OT[ht][:], rt[:])

        def av_mm_t(avA, avB, pb, jt, ic, at, start, stop):
            cc = slice(ic * NI, (ic + 1) * NI)
            nc.tensor.matmul(
                avA[:, cc], Vaug[jt][:, pb : pb + VA],
                at[:, 0:NI], start=start, stop=stop,
            )
            nc.tensor.matmul(
                avB[:, cc], Vaug[jt][:, pb + VA : pb + PW],
                at[:, NI:S], start=start, stop=stop,
            )

        pending = None
        av_pending = None
        for ht in range(HT):
            pb = ht * PW
            avA = ps.tile([VA, S], F32, tag="avA", bufs=1, name="avA")
            avB = ps.tile([P, S], F32, tag="avB", bufs=1, name="avB")

            def av_mm(jt, ic, at, start, stop, avA=avA, avB=avB, pb=pb):
                av_mm_t(avA, avB, pb, jt, ic, at, start, stop)

            if ht == 0:
                # Head 0 carries the V projection and the remaining Q/K
                # projections. Phase 1 (scores jt0/jt1) fills the PE while
                # xv/wv stream in; phase 2 runs per jt: V-proj tile, the
                # Q(jt)/K(jt) projections (their weights land mid-phase),
                # scores jt+2, and the 1-jt-behind AV so nothing in-order
                # waits on a copy that just issued. All-ScalarE exp.
                ht0_at = {}
                for jt in (0, 1):
                    for ic in range(IC):
                        sc = scores_tile(ht, jt, ic)
                        ht0_at[(jt, ic)] = exp_tile(sc, jt, "S", long_lived=True)
                for jt in range(ST):
                    v_proj_tile(jt)
                    if jt >= 1:
                        proj_qk(xq3, wq4, QT[jt], jt, None if not has_bias else bqT)
                        proj_qk(xk3, wk4, KT[jt], jt, None if not has_bias else bkT)
                    if jt < ST - 2:
                        for ic in range(IC):
                            sc = scores_tile(ht, jt + 2, ic)
                            ht0_at[(jt + 2, ic)] = exp_tile(
                                sc, jt + 2, "S" if jt % 2 == 0 else "H",
                                long_lived=False,
                            )
                    if jt >= 1:
                        for ic in range(IC):
                            av_mm(
                                jt - 1, ic, ht0_at.pop((jt - 1, ic)),
                                start=(jt - 1 == 0), stop=False,
                            )
                    if jt == ST - 1:
                        wvp.release()
                        wop = tc.alloc_tile_pool(name="wop", bufs=1)
                        wo = wop.tile([P, HT * H], BF, tag="wo", name="wo")
                        nc.sync.dma_start(out=wo[:, 0 : 4 * H], in_=wo_d[:, 0 : 4 * H])
                        nc.sync.dma_start(out=wo[:, 4 * H :], in_=wo_d[:, 4 * H :])
                for ic in range(IC):
                    av_mm(
                        ST - 1, ic, ht0_at.pop((ST - 1, ic)),
                        start=False, stop=True,
                    )
            else:
                # Boundary: the previous head's four PSUM evacuation reads
                # all ride ScalarE (whose first exp isn't needed until ~5us
                # into the head), so the Vector queue opens with the four
                # Schraudolph TS ops that recycle the 'big' score bufs. The
                # bitcast-copy halves (GpSimd+Vector) are emitted after all
                # four TS so they don't delay buf recycling either.
                pht, pavA, pavB, ppb, pats = av_pending
                av_pending = None
                pending = emit_evac(pht, pavA, pavB)
                ats = {}
                for jt in (0, 1):
                    for ic in range(IC):
                        sc = scores_tile(ht, jt, ic)
                        ats[(jt, ic)] = exp_tile(sc, jt, "H", long_lived=True)
                pending = emit_dsg(pending)
                # 2-tile lookahead: scores run two tiles ahead of their AV so
                # each exp has ~3.4us of PE work to hide behind. jt0/jt1 AVs
                # are deferred to jt6 so the head still ends on jt7's stop.
                # jt2..5 exp on ScalarE, jt6/jt7 on the VectorE H path.
                prevq = []
                for jt in range(2, ST):
                    for ic in range(IC):
                        sc = scores_tile(ht, jt, ic)
                        at = exp_tile(
                            sc, jt, "S" if jt <= 5 else "H", long_lived=False
                        )
                        prevq.append((jt, ic, at))
                        if len(prevq) > 2:
                            t = prevq.pop(0)
                            av_mm(t[0], t[1], t[2], start=(t[0] == 2), stop=False)
                    if jt == 5 and pending is not None:
                        emit_normalize(pending)
                        pending = None
                    if jt == 6:
                        for djt in (1, 0):
                            for ic2 in range(IC):
                                av_mm(
                                    djt, ic2, ats[(djt, ic2)],
                                    start=False, stop=False,
                                )
                for t in prevq:
                    av_mm(t[0], t[1], t[2], start=False, stop=True)
            av_pending = (ht, avA, avB, pb, None)

        pht, pavA, pavB, ppb, pats = av_pending
        # Final head: denominator staging first - the normalize chain gates
        # the output projection's kt7 matmuls.
        dcf = sb.tile([2, S], F32, tag="dcf", bufs=2, name="dcf")
        nc.scalar.copy(dcf[0:1, :], pavB[0:1, :])
        dsg = sb.tile([VA, S], F32, tag="dsg", bufs=2, name="dsg")
        nc.vector.tensor_copy(dsg[HD:VA, :], pavA[HD:VA, :])
        nc.sync.dma_start(out=dcf[1:2, :], in_=dsg[HD:VA, :])
        nc.scalar.copy(OT[pht][0:HD, :], pavA[0:HD, :])
        nc.scalar.copy(OT[pht][HD:P, :], pavB[HD:P, :])
        # rt in the (now idle) avB bank so the output projection's 'big'
        # rotation doesn't wait on this chain.
        emit_normalize((pht, dcf), rt_tag="avB", rt_bufs=1)

        # ---------- output projection (wo preloaded during attention) ------
        wo3 = wo.rearrange("p (kt c) -> p kt c", c=H)
        for st in range(ST):
            py = ps.tile([P, S], F32, tag="big", bufs=2, name="py")
            for kt in range(HT):
                os_ = OT[kt][:, st * P : (st + 1) * P]
                nc.tensor.matmul(
                    py[:, 0:NI], os_, wo3[:, kt, 0:NI],
                    start=(kt == 0), stop=(kt == HT - 1),
                )
                nc.tensor.matmul(
                    py[:, NI:S], os_, wo3[:, kt, NI:S],
                    start=(kt == 0), stop=(kt == HT - 1),
                )
            ysb = sb.tile([P, S], BF, tag="ysb", bufs=2, name="ysb")
            halves = [(0, P)] if st < ST - 1 else [(0, HD), (HD, P)]
            for h0, h1 in halves:
                if has_bias:
                    nc.vector.tensor_add(ysb[h0:h1, :], py[h0:h1, :], bob[h0:h1, :])
                else:
                    nc.scalar.copy(ysb[h0:h1, :], py[h0:h1, :])
                nc.sync.dma_start(
                    out=y_d[st * P + h0 : st * P + h1, :], in_=ysb[h0:h1, :]
                )
        wop.release()

    nc.compile()
    return nc


def _bf(x):
    return np.ascontiguousarray(np.asarray(x, np.float32), dtype=ml_dtypes.bfloat16)


def _f8(x):
    return np.ascontiguousarray(
        np.asarray(x, np.float32), dtype=ml_dtypes.float8_e4m3
    )


def _f32(x):
    return np.ascontiguousarray(x, dtype=np.float32)


def _tile_rows(xT):
    # [HT*P, C] -> [P, HT*C]: [p, kt*C+c] = xT[kt*P+p, c]
    C = xT.shape[1]
    return np.ascontiguousarray(
        xT.reshape(HT, P, C).transpose(1, 0, 2).reshape(P, HT * C)
    )


def _tile_wqk(W):
    # [p, ot*H + kt*128 + c] = WS * W.T[kt*128+p, ot*128+c]
    wT = np.asarray(W, np.float32).T.reshape(HT, P, HT, P)
    return _f8(WS * wT.transpose(1, 2, 0, 3).reshape(P, HT * H))


def prep_inputs(query, key, value, mask, Wq, bq, Wk, bk, Wv, bv, Wo, bo, has_bias):
    wq = _tile_wqk(Wq)
    wk = _tile_wqk(Wk)
    wv = _bf(_tile_rows(np.asarray(Wv, np.float32).T))
    wo = _bf(_tile_rows(np.asarray(Wo, np.float32).T))
    sel2 = np.zeros((2, P), np.float32)
    sel2[0, HD:P] = 1.0  # row 0 = 1/D_B -> head B partitions
    sel2[1, 0:HD] = 1.0  # row 1 = 1/D_A -> head A partitions
    sel2 = _bf(sel2)

    com = {"wq": wq, "wk": wk, "wv": wv, "wo": wo, "sel2": sel2}
    if has_bias:
        com["bqT"] = _f32(WS * np.asarray(bq, np.float32).reshape(HT, P).T)
        com["bkT"] = _f32(WS * np.asarray(bk, np.float32).reshape(HT, P).T)
        com["bvb"] = _bf(np.broadcast_to(np.asarray(bv, np.float32), (P, H)))
        com["bob"] = _f32(np.broadcast_to(np.asarray(bo, np.float32), (P, H)))

    in_maps = []
    for b in range(B):
        mb = np.asarray(mask[b]).reshape(ST, P).T  # [P, ST] bool, True=masked
        in_maps.append(
            {
                "xq": _f8(_tile_rows(np.asarray(query[b], np.float32).T)),
                "xk": _f8(_tile_rows(np.asarray(key[b], np.float32).T)),
                "xv": _bf(_tile_rows(np.asarray(value[b], np.float32).T)),
                "maskb": _f32(np.where(mb, NEG, np.float32(0.0))),
                "sbias": _f32(
                    np.where(mb, np.float32(SCH16_MASKED), np.float32(B16))
                ),
                **com,
            }
        )
    return in_maps


def kernel(
    query, key, value, mask, seq_mask, Wq, bq, Wk, bk, Wv, bv, Wo, bo, **run_kwargs
):
    assert int(np.asarray(seq_mask)) == 0, "causal masking not implemented"
    has_bias = any(bool(np.any(np.asarray(b))) for b in (bq, bk, bv, bo))
    key_ = ("nc", has_bias)
    if key_ not in _CACHE:
        _CACHE[key_] = build_program(has_bias)
    nc = _CACHE[key_]
    in_maps = prep_inputs(
        query, key, value, mask, Wq, bq, Wk, bk, Wv, bv, Wo, bo, has_bias
    )
    res = run_bass_kernel_spmd(nc, in_maps, list(range(B)), **run_kwargs)
    out = np.stack(
        [np.asarray(res.results[b]["y"], dtype=np.float32) for b in range(B)], axis=0
    )
    if run_kwargs:
        _CACHE["last_result"] = res
    return out
